# revision 6
# baseline (speedup 1.0000x reference)
"""Causal self-attention for trn2, 8 NeuronCores.

Problem: x[4,2048,1024] @ w_qkv[1024,3072] -> causal MHA (16 heads, d=64)
-> @ w_out[1024,1024].

Sharding: core c handles batch b=c%4 and heads hbase=8*(c//4)..hbase+8
(data parallel on B x tensor parallel on heads). Each core computes the
partial out-projection y_c = att_slice @ w_out[slice]; the host sums the
two partials per batch.

All matmuls run in float32r (PE full-rate fp32, ~1.5e-4 relative) with
operands rounded by their DVE/ACT producers. Softmax denominators come from
a fused ones-column in the AV matmul ([V|1]^T w^T row 64); causal masking
skips above-diagonal tiles entirely and applies one gpsimd affine_select
per diagonal 128x128 block after the exp. x is transposed on-chip via PE
transpose-mode, processed in T-quarters to fit SBUF.
"""

import sys

for p in ("/opt/trn_rl_repo", "/opt/pypackages"):
    if p not in sys.path:
        sys.path.insert(0, p)

import contextlib

import numpy as np

import concourse.bass as bass
import concourse.mybir as mybir
import concourse.tile as tile
from concourse import bacc
from concourse.bass_utils import run_bass_kernel_spmd
from concourse.masks import make_identity

F32 = mybir.dt.float32
F32R = mybir.dt.float32r
EXP = mybir.ActivationFunctionType.Exp

T = 2048          # sequence length
C = 1024          # model dim
HC = 8            # heads per core
D = 64            # head dim
NG = 4            # head-groups of 2 per core
NCT = C // 128    # 8 contraction tiles
NTT = T // 128    # 16 token tiles
NQB = 4           # q blocks of 512
SCALE = 0.125     # 1/sqrt(D)


def build_nc():
    nc = bacc.Bacc("TRN2", target_bir_lowering=False, debug=False)

    x_d = nc.dram_tensor("x", [T, C], F32, kind="ExternalInput")
    wq_d = nc.dram_tensor("wq", [C, 512], F32, kind="ExternalInput")
    wk_d = nc.dram_tensor("wk", [C, 512], F32, kind="ExternalInput")
    wv_d = nc.dram_tensor("wv", [C, 512], F32, kind="ExternalInput")
    wo_d = nc.dram_tensor("wo", [512, C], F32, kind="ExternalInput")
    y_d = nc.dram_tensor("y", [T, C], F32, kind="ExternalOutput")

    with tile.TileContext(nc) as tc, contextlib.ExitStack() as ctx:
        persist = ctx.enter_context(tc.tile_pool(name="persist", bufs=1))

        ident = persist.tile([128, 128], F32)
        make_identity(nc, ident)

        qT = [persist.tile([128, T], F32R, tag=f"qT{g}", name=f"qT{g}") for g in range(NG)]
        kT = [persist.tile([128, T], F32R, tag=f"kT{g}", name=f"kT{g}") for g in range(NG)]
        # V with fused ones column: [128part, tt, head, 65]
        V = persist.tile([128, NTT, HC, 65], F32R, tag="V")

        # ---------------- phases A+B: transpose x & project, per T-quarter --
        with (
            tc.tile_pool(name="pw", bufs=1) as pw,
            tc.tile_pool(name="pab", bufs=2) as pab,
            tc.tile_pool(name="pxt", bufs=1) as pxt,
            tc.tile_pool(name="pab_ps", bufs=2, space="PSUM") as pab_ps,
        ):
            # wv: load + round once (resident); wq/wk streamed per (qtr, g)
            wv_r = pw.tile([128, NCT, 512], F32R, tag="wv_r")
            wvv = wv_d.ap().rearrange("(ct p) m -> p ct m", p=128)
            for h2 in range(2):
                wvs = pab.tile([128, 4, 512], F32, tag="wv_stage", bufs=1)
                nc.sync.dma_start(out=wvs, in_=wvv[:, 4 * h2:4 * h2 + 4, :])
                nc.vector.tensor_copy(wv_r[:, 4 * h2:4 * h2 + 4, :], wvs)

            # ones column of V
            ones_f32 = pw.tile([128, NTT, HC], F32, tag="ones")
            nc.vector.memset(ones_f32, 1.0)
            nc.vector.tensor_copy(V[:, :, :, 64], ones_f32)

            # dummy transpose absorbs the gpsimd (make_identity) tick
            dtp = pab_ps.tile([128, 512], F32, tag="tp")
            nc.tensor.transpose(dtp[:, 0:128], ident, ident)
            ddum = pab.tile([128, 128], F32, tag="dummy", bufs=1)
            nc.vector.tensor_copy(ddum, dtp[:, 0:128])

            for qtr in range(4):
                xTq = [pxt.tile([128, 512], F32R, tag=f"xTq{ct}", name=f"xTq{ct}") for ct in range(NCT)]
                # transpose this quarter of x
                for j in range(4):  # token tile within quarter
                    row0 = qtr * 512 + j * 128
                    x_nat = pab.tile([128, C], F32, tag="x_nat")
                    nc.sync.dma_start(out=x_nat, in_=x_d.ap()[row0:row0 + 128, :])
                    for ctq in range(2):
                        tp = pab_ps.tile([128, 512], F32, tag="tp")
                        for m in range(4):
                            ct = 4 * ctq + m
                            nc.tensor.transpose(
                                tp[:, m * 128:(m + 1) * 128],
                                x_nat[:, ct * 128:(ct + 1) * 128],
                                ident,
                            )
                        for m in range(4):
                            ct = 4 * ctq + m
                            nc.vector.tensor_copy(
                                xTq[ct][:, j * 128:(j + 1) * 128],
                                tp[:, m * 128:(m + 1) * 128],
                            )
                # qT/kT for this quarter (weights streamed per group)
                for g in range(NG):
                    for wdram, dst in ((wq_d, qT[g]), (wk_d, kT[g])):
                        ws = pab.tile([128, NCT, 128], F32, tag="w_stage")
                        nc.sync.dma_start(
                            out=ws,
                            in_=wdram.ap()[:, g * 128:(g + 1) * 128].rearrange(
                                "(ct p) m -> p ct m", p=128
                            ),
                        )
                        w_r = pab.tile([128, NCT, 128], F32R, tag="w_r")
                        nc.vector.tensor_copy(w_r, ws)
                        ps = pab_ps.tile([128, 512], F32, tag="ps_qk")
                        for ct in range(NCT):
                            nc.tensor.matmul(
                                ps,
                                w_r[:, ct, :],
                                xTq[ct],
                                start=(ct == 0),
                                stop=(ct == NCT - 1),
                            )
                        nc.vector.tensor_copy(
                            dst[:, qtr * 512:(qtr + 1) * 512], ps
                        )
                # V for this quarter
                for j in range(4):
                    tt = qtr * 4 + j
                    ps = pab_ps.tile([128, 512], F32, tag="ps_v")
                    for ct in range(NCT):
                        nc.tensor.matmul(
                            ps,
                            xTq[ct][:, j * 128:(j + 1) * 128],
                            wv_r[:, ct, :],
                            start=(ct == 0),
                            stop=(ct == NCT - 1),
                        )
                    for h in range(HC):
                        nc.vector.tensor_copy(
                            V[:, tt, h, 0:64], ps[:, h * 64:(h + 1) * 64]
                        )

        # ---------------- phase C: attention --------------------------------
        attT = [persist.tile([128, T], F32R, tag=f"attT{g}", name=f"attT{g}") for g in range(NG)]

        with (
            tc.tile_pool(name="phc_sb", bufs=3) as phc_sb,
            tc.tile_pool(name="phc_sb2", bufs=2) as phc_sb2,
            tc.tile_pool(name="phc_psA", bufs=2, space="PSUM") as phc_psA,
            tc.tile_pool(name="phc_psB", bufs=1, space="PSUM") as phc_psB,
            tc.tile_pool(name="phc_psav", bufs=1, space="PSUM") as phc_psav,
            tc.tile_pool(name="phc_dram", bufs=4, space="DRAM") as phc_dram,
        ):
            for g in range(NG):
                hA, hB = 2 * g, 2 * g + 1
                for qb in range(NQB):
                    nkt = 4 * (qb + 1)
                    av_A = phc_psav.tile([65, 512], F32, tag="av_A")
                    av_B = phc_psav.tile([65, 512], F32, tag="av_B")
                    for kp in range(nkt // 2):
                        sA = phc_psA.tile([128, 1024], F32, tag="sA")
                        sB = phc_psB.tile([128, 1024], F32, tag="sB")
                        for i in range(2):
                            kt = 2 * kp + i
                            nc.tensor.matmul(
                                sA[:, i * 512:(i + 1) * 512],
                                kT[g][0:64, kt * 128:(kt + 1) * 128],
                                qT[g][0:64, qb * 512:(qb + 1) * 512],
                                start=True, stop=True,
                                tile_position=(0, 0),
                            )
                            nc.tensor.matmul(
                                sB[:, i * 512:(i + 1) * 512],
                                kT[g][64:128, kt * 128:(kt + 1) * 128],
                                qT[g][64:128, qb * 512:(qb + 1) * 512],
                                start=True, stop=True,
                                tile_position=(64, 0),
                            )
                        wT_A = phc_sb.tile([128, 1024], F32R, tag="wT_A")
                        wT_B = phc_sb.tile([128, 1024], F32R, tag="wT_B")
                        nc.scalar.activation(wT_A, sA, EXP, scale=SCALE)
                        nc.scalar.activation(wT_B, sB, EXP, scale=SCALE)
                        # causal mask on diagonal tiles (kt in [4qb, 4qb+3])
                        for i in range(2):
                            kt = 2 * kp + i
                            j = kt - 4 * qb
                            if j >= 0:
                                ncols = 128 * j + 128
                                for wT in (wT_A, wT_B):
                                    nc.gpsimd.affine_select(
                                        out=wT[:, i * 512:i * 512 + ncols],
                                        in_=wT[:, i * 512:i * 512 + ncols],
                                        compare_op=mybir.AluOpType.is_ge,
                                        fill=0.0,
                                        base=-128 * j,
                                        pattern=[[1, ncols]],
                                        channel_multiplier=-1,
                                    )
                        for i in range(2):
                            kt = 2 * kp + i
                            nc.tensor.matmul(
                                av_A,
                                V[:, kt, hA, :],
                                wT_A[:, i * 512:(i + 1) * 512],
                                start=(kt == 0), stop=(kt == nkt - 1),
                            )
                            nc.tensor.matmul(
                                av_B,
                                V[:, kt, hB, :],
                                wT_B[:, i * 512:(i + 1) * 512],
                                start=(kt == 0), stop=(kt == nkt - 1),
                            )
                    # normalize: recip of denom row 64, DRAM-bounce partition
                    # broadcast, multiply into attT
                    rec = phc_sb2.tile([65, 1024], F32, tag="rec")
                    nc.vector.reciprocal(rec[64:65, 0:512], av_A[64:65, :])
                    nc.vector.reciprocal(rec[64:65, 512:1024], av_B[64:65, :])
                    rec_d = phc_dram.tile([1, 1024], F32, tag="rec_d")
                    nc.sync.dma_start(out=rec_d, in_=rec[64:65, :])
                    rep = phc_sb2.tile([64, 1024], F32, tag="rep")
                    nc.sync.dma_start(
                        out=rep,
                        in_=bass.AP(rec_d.tensor, rec_d.offset, [[0, 64], [1, 1024]]),
                    )
                    nc.vector.tensor_mul(
                        attT[g][0:64, qb * 512:(qb + 1) * 512],
                        av_A[0:64, :],
                        rep[:, 0:512],
                    )
                    tmpB = phc_sb2.tile([64, 512], F32R, tag="tmpB")
                    nc.vector.tensor_mul(tmpB, av_B[0:64, :], rep[:, 512:1024])
                    nc.sync.dma_start(
                        out=attT[g][64:128, qb * 512:(qb + 1) * 512], in_=tmpB
                    )

        # ---------------- phase D: out projection ----------------------------
        with (
            tc.tile_pool(name="phd_sb", bufs=2) as phd_sb,
            tc.tile_pool(name="phd_ps", bufs=3, space="PSUM") as phd_ps,
        ):
            wo_f = phd_sb.tile([128, NG, C], F32, tag="wo_f")
            nc.sync.dma_start(
                out=wo_f, in_=wo_d.ap().rearrange("(g p) c -> p g c", p=128)
            )
            wo_r = phd_sb.tile([128, NG, C], F32R, tag="wo_r")
            nc.vector.tensor_copy(wo_r, wo_f)
            for qt in range(NTT):
                ps = phd_ps.tile([128, 1024], F32, tag="ps_y")
                for g in range(NG):
                    for half in range(2):
                        nc.tensor.matmul(
                            ps[:, half * 512:(half + 1) * 512],
                            attT[g][:, qt * 128:(qt + 1) * 128],
                            wo_r[:, g, half * 512:(half + 1) * 512],
                            start=(g == 0),
                            stop=(g == NG - 1),
                        )
                y_sb = phd_sb.tile([128, C], F32, tag="y_sb")
                nc.vector.tensor_copy(y_sb, ps)
                nc.sync.dma_start(
                    out=y_d.ap()[qt * 128:(qt + 1) * 128, :], in_=y_sb
                )

    nc.compile()
    return nc


_NC_CACHE = None


def _get_nc():
    global _NC_CACHE
    if _NC_CACHE is None:
        _NC_CACHE = build_nc()
    return _NC_CACHE


def kernel(x, w_qkv, w_out, _trace=False):
    B = x.shape[0]
    x = np.ascontiguousarray(x, dtype=np.float32)
    w_qkv = np.ascontiguousarray(w_qkv, dtype=np.float32)
    w_out = np.ascontiguousarray(w_out, dtype=np.float32)

    nc = _get_nc()
    in_maps = []
    for core in range(8):
        b = core % B
        hbase = (core // B) * HC
        lo, hi = hbase * D, hbase * D + HC * D
        in_maps.append({
            "x": x[b],
            "wq": np.ascontiguousarray(w_qkv[:, lo:hi]),
            "wk": np.ascontiguousarray(w_qkv[:, C + lo:C + hi]),
            "wv": np.ascontiguousarray(w_qkv[:, 2 * C + lo:2 * C + hi]),
            "wo": np.ascontiguousarray(w_out[lo:hi, :]),
        })

    res = run_bass_kernel_spmd(nc, in_maps, core_ids=list(range(8)), trace=_trace)
    ys = [r["y"] for r in res.results]
    out = np.empty((B, T, C), dtype=np.float32)
    for b in range(B):
        out[b] = ys[b] + ys[b + B]
    if _trace:
        return out, res
    return out
